# revision 1
# baseline (speedup 1.0000x reference)
"""Trainium2 Bass kernel for nn_Attention (qkv+BN -> biased softmax attention -> gelu -> proj+BN).

Sharding: data-parallel over batch B=128 across 8 NeuronCores (16 batches each).
BatchNorm (training-mode) statistics are all-reduced across cores (tiny collectives).

Per-core layout strategy:
  - x shard (5120, 256) f32, transposed on PE to xT [c, row].
  - qkv computed as [h_dim, row] with HOST-permuted Wqkv so the 1536 h-dims land
    as [q(8x32) | k(8x32) | v(8x128)] -> chunks 0-1 q, 2-3 k, 4-11 v (one v chunk per head).
  - softmax(s+bias) = exp(s)*exp(bias) / rowsum: exp(bias) gathered on host (replicated),
    multiplied on DVE; rowsums via ones-matmul on PE; softmax divide deferred past AV.
  - attention head-major; AV output overwrites that head's v tile in SBUF.
  - divide+gelu+proj+BN2 in a final phase (single activation-table switch to gelu).
"""
import os
import numpy as np
import ml_dtypes

import concourse.bass as bass
import concourse.tile as tile
from concourse import bacc, mybir
from concourse.bass_utils import run_bass_kernel_spmd

NCORES = int(os.environ.get("KERN_NCORES", "8"))
DBG_STOP = os.environ.get("KERN_STOP", "")   # "A" or "B" to stop early
BSUB = int(os.environ.get("KERN_BSUB", "4"))
TRACE_SIM = os.environ.get("KERN_TRACE_SIM", "") == "1"
B, N, C = 128, 320, 256
NH, DK, DV = 8, 32, 128
H = NH * (2 * DK + DV)       # 1536
DH = NH * DV                 # 1024
BL = B // 8                  # 16 batches/core (fixed shard size)
R = BL * N                   # 5120 rows/core
NT = B * N                   # 40960 global rows
EPS = 1e-5
SCALE = DK ** -0.5
FP = mybir.dt.float32
FR = mybir.dt.float32r
BF = mybir.dt.bfloat16

NHC = H // 128               # 12 h-chunks
NRB = R // 512               # 10 row blocks of 512
MCS = [128, 128, 64]         # chunking of N=320
AF = mybir.ActivationFunctionType
OP = mybir.AluOpType


def build_program():
    nc = bacc.Bacc("TRN2", target_bir_lowering=False, debug=False,
                   enable_asserts=False, num_devices=NCORES)
    x_d = nc.dram_tensor("x", [R, C], FP, kind="ExternalInput").ap()
    wqkvT_d = nc.dram_tensor("wqkvT", [C, H], BF, kind="ExternalInput").ap()
    wprojT_d = nc.dram_tensor("wprojT", [DH, C], BF, kind="ExternalInput").ap()
    eb_d = nc.dram_tensor("eb", [NH, N, N], BF, kind="ExternalInput").ap()
    g1_d = nc.dram_tensor("g1c", [128, NHC], FP, kind="ExternalInput").ap()
    b1_d = nc.dram_tensor("b1c", [128, NHC], FP, kind="ExternalInput").ap()
    g2_d = nc.dram_tensor("g2", [1, C], FP, kind="ExternalInput").ap()
    b2_d = nc.dram_tensor("b2", [1, C], FP, kind="ExternalInput").ap()
    id_d = nc.dram_tensor("ident", [128, 128], FP, kind="ExternalInput").ap()
    y_d = nc.dram_tensor("y", [R, C], FP, kind="ExternalOutput").ap()

    with tile.TileContext(nc, trace_sim=TRACE_SIM) as tc:
        with tc.tile_pool(name="const", bufs=1) as constp, \
             tc.tile_pool(name="qkv", bufs=1) as qkvp, \
             tc.tile_pool(name="dram", bufs=1, space="DRAM") as dramp, \
             tc.tile_pool(name="stat", bufs=1) as statp:

            # ---- constants ----
            wprojT_sb = constp.tile([128, NH * C], BF)     # 8 d-chunks side by side
            for dc in range(NH):
                nc.sync.dma_start(wprojT_sb[:, dc * C:(dc + 1) * C],
                                  wprojT_d[dc * 128:(dc + 1) * 128, :])
            g1_sb = constp.tile([128, NHC], FP)
            b1_sb = constp.tile([128, NHC], FP)
            g2_sb = constp.tile([1, C], FP)
            b2_sb = constp.tile([1, C], FP)
            id_sb = constp.tile([128, 128], FP)
            nc.sync.dma_start(g1_sb[:], g1_d[:])
            nc.sync.dma_start(b1_sb[:], b1_d[:])
            nc.sync.dma_start(g2_sb[:], g2_d[:])
            nc.sync.dma_start(b2_sb[:], b2_d[:])
            nc.sync.dma_start(id_sb[:], id_d[:])
            ones_c = constp.tile([128, 1], BF)             # ones column (bf16 matmuls)
            nc.vector.memset(ones_c[:], 1.0)
            ones_cf = constp.tile([128, 1], FP)            # ones column f32
            nc.vector.memset(ones_cf[:], 1.0)
            ones_r = constp.tile([128, 128], FP)           # ones (broadcast outers)
            nc.vector.memset(ones_r[:], 1.0)
            ones_rb = constp.tile([128, 128], BF)          # bf16 ones (outers)
            nc.vector.memset(ones_rb[:], 1.0)

            # ---- persistent big buffers ----
            qkv_sb = [qkvp.tile([128, R], BF, tag=f"qkv{i}", name=f"qkv{i}")
                      for i in range(NHC)]
            ssum = statp.tile([128, NHC * NRB], FP)
            ssq = statp.tile([128, NHC * NRB], FP)

            # ========== Phase A: x^T, qkv matmul, BN1 stats ==========
            with tc.tile_pool(name="xa", bufs=1) as xp, \
                 tc.tile_pool(name="pa", bufs=4, space="PSUM") as pap, \
                 tc.tile_pool(name="sc", bufs=4) as scp:
                xT_sb = [xp.tile([128, R], BF, tag=f"xT{cc}", name=f"xT{cc}")
                         for cc in range(2)]
                with tc.tile_pool(name="xr", bufs=8) as xrp:
                    for rc in range(40):
                        xt_t = xrp.tile([128, C], FP, tag="x", name="xt_t")
                        nc.sync.dma_start(xt_t[:], x_d[rc * 128:(rc + 1) * 128, :])
                        for cc in range(2):
                            pt = pap.tile([128, 128], FP, tag="tp")
                            nc.tensor.transpose(
                                pt[:], xt_t[:, cc * 128:(cc + 1) * 128], id_sb[:])
                            nc.scalar.copy(xT_sb[cc][:, rc * 128:(rc + 1) * 128],
                                           pt[:])
                wq_sb = [xp.tile([128, H], BF, tag=f"wq{cc}", name=f"wq{cc}")
                         for cc in range(2)]
                for cc in range(2):
                    nc.sync.dma_start(wq_sb[cc][:], wqkvT_d[cc * 128:(cc + 1) * 128, :])

                for rb in range(NRB):
                    for hc in range(NHC):
                        pq = pap.tile([128, 512], FP, tag="pq")
                        for cc in range(2):
                            nc.tensor.matmul(
                                pq[:],
                                wq_sb[cc][:, hc * 128:(hc + 1) * 128],
                                xT_sb[cc][:, rb * 512:(rb + 1) * 512],
                                start=(cc == 0), stop=(cc == 1))
                        col = hc * NRB + rb
                        nc.vector.tensor_scalar(
                            qkv_sb[hc][:, rb * 512:(rb + 1) * 512], pq[:],
                            1.0, 0.0, OP.mult, OP.add,
                            accum_out=ssum[:, col:col + 1])
                        sq = scp.tile([128, 512], BF, tag="sq")
                        nc.scalar.activation(
                            sq[:], qkv_sb[hc][:, rb * 512:(rb + 1) * 512],
                            AF.Square, accum_out=ssq[:, col:col + 1])

            # stats partials -> allreduce -> BN1 affine coefficients
            stats = statp.tile([128, 2 * NHC], FP)
            for hc in range(NHC):
                nc.vector.tensor_reduce(
                    stats[:, hc:hc + 1], ssum[:, hc * NRB:(hc + 1) * NRB],
                    mybir.AxisListType.X, OP.add)
                nc.vector.tensor_reduce(
                    stats[:, NHC + hc:NHC + hc + 1], ssq[:, hc * NRB:(hc + 1) * NRB],
                    mybir.AxisListType.X, OP.add)
            bounce_i = dramp.tile([128, 2 * NHC], FP, tag="b1i")
            bounce_o = dramp.tile([128, 2 * NHC], FP, tag="b1o")
            nc.sync.dma_start(bounce_i[:], stats[:])
            nc.gpsimd.collective_compute(
                "AllReduce", OP.add,
                replica_groups=[list(range(NCORES))],
                ins=[bounce_i.opt()], outs=[bounce_o.opt()])
            statsg = statp.tile([128, 2 * NHC], FP)
            nc.sync.dma_start(statsg[:], bounce_o[:])

            mean1 = statp.tile([128, NHC], FP)
            var1 = statp.tile([128, NHC], FP)
            tmp1 = statp.tile([128, NHC], FP)
            alpha1 = statp.tile([128, NHC], FP)
            beta1 = statp.tile([128, NHC], FP)
            nc.vector.tensor_scalar(mean1[:], statsg[:, 0:NHC], 1.0 / NT, None, OP.mult)
            nc.vector.tensor_scalar(var1[:], statsg[:, NHC:2 * NHC], 1.0 / NT, None, OP.mult)
            nc.vector.tensor_tensor(tmp1[:], mean1[:], mean1[:], OP.mult)
            nc.vector.tensor_tensor(var1[:], var1[:], tmp1[:], OP.subtract)
            nc.vector.tensor_scalar(var1[:], var1[:], EPS, None, OP.add)
            nc.scalar.activation(tmp1[:], var1[:], AF.Ln)
            nc.scalar.activation(var1[:], tmp1[:], AF.Exp, scale=-0.5)   # rstd
            nc.vector.tensor_tensor(alpha1[:], g1_sb[:], var1[:], OP.mult)
            nc.vector.tensor_tensor(beta1[:], mean1[:], alpha1[:], OP.mult)
            nc.vector.tensor_tensor(beta1[:], b1_sb[:], beta1[:], OP.subtract)
            for hc in range(4):                       # q,k only; v folded into gelu
                nc.vector.tensor_scalar(
                    qkv_sb[hc][:], qkv_sb[hc][:],
                    alpha1[:, hc:hc + 1], beta1[:, hc:hc + 1], OP.mult, OP.add)

            if DBG_STOP not in ("A",):
                # ========== Phase B: attention, head-major ==========
                # rowsums packed 4 batches per [128,N] tile at partition rows 0/32/64/96
                with tc.tile_pool(name="pb", bufs=2, space="PSUM") as pbp, \
                     tc.tile_pool(name="pr", bufs=2, space="PSUM") as prp, \
                     tc.tile_pool(name="prbc", bufs=2, space="PSUM") as prbc, \
                     tc.tile_pool(name="eb2", bufs=9) as ep, \
                     tc.tile_pool(name="rrp", bufs=8) as rrp, \
                     tc.tile_pool(name="sc2", bufs=4) as scp2, \
                     tc.tile_pool(name="ebp", bufs=1) as ebp, \
                     tc.tile_pool(name="vr", bufs=18) as vp:
                    eb_sb = []                                 # [h][mc] -> [<=128, 320]
                    for h in range(NH):
                        row = []
                        for mc in range(3):
                            t = ebp.tile([128, N], BF, tag=f"eb{h}_{mc}",
                                         name=f"eb{h}_{mc}")
                            mb = 64 if mc == 2 else 0
                            nc.sync.dma_start(
                                t[mb:mb + MCS[mc], :],
                                eb_d[h, 128 * mc:128 * mc + MCS[mc], :])
                            row.append(t)
                        eb_sb.append(row)
                    for h in range(NH):
                        qc, qr = h // 4, 32 * (h % 4)
                        r_recip = []
                        for b in range(BL):
                            if b % 4 == 0:
                                rp = prp.tile([128, N], FP, tag="r", name="rp")
                            q_ap = qkv_sb[qc][qr:qr + 32, b * N:(b + 1) * N]
                            k_ap = qkv_sb[2 + qc][qr:qr + 32, b * N:(b + 1) * N]
                            e_tiles = []
                            for mc in range(3):
                                ms = MCS[mc]
                                mb = 64 if mc == 2 else 0
                                ps = pbp.tile([128, N], FP, tag="ps")
                                nc.tensor.matmul(
                                    ps[mb:mb + ms, :],
                                    k_ap[:, 128 * mc:128 * mc + ms],
                                    q_ap[:], tile_position=(qr, mb))
                                et = ep.tile([128, N], BF, tag="et")
                                nc.scalar.activation(et[mb:mb + ms, :], ps[mb:mb + ms, :],
                                                     AF.Exp, scale=SCALE)
                                nc.gpsimd.tensor_tensor(
                                    et[mb:mb + ms, :], et[mb:mb + ms, :],
                                    eb_sb[h][mc][mb:mb + ms, :], OP.mult)
                                e_tiles.append(et)
                            rrow = 32 * (b % 4)
                            for mc in range(3 if BSUB >= 2 else 0):
                                ms = MCS[mc]
                                mb = 64 if mc == 2 else 0
                                nc.tensor.matmul(
                                    rp[rrow:rrow + 1, :], ones_c[mb:mb + ms, 0:1],
                                    e_tiles[mc][mb:mb + ms, :],
                                    start=(mc == 0), stop=(mc == 2),
                                    tile_position=(mb, rrow))
                            av = pbp.tile([128, N], FP, tag="av")
                            for mc in range(3 if BSUB >= 3 else 0):
                                ms = MCS[mc]
                                mb = 64 if mc == 2 else 0
                                c0 = b * N + (192 if mc == 2 else 128 * mc)
                                vt = vp.tile([128, 128], BF, tag="vt")
                                nc.sync.dma_start_transpose(
                                    vt[:], qkv_sb[4 + h][:, c0:c0 + 128])
                                nc.tensor.matmul(
                                    av[:], vt[mb:mb + ms, :], e_tiles[mc][mb:mb + ms, :],
                                    start=(mc == 0), stop=(mc == 2),
                                    tile_position=(mb, 0))
                            if BSUB < 3: av = None
                            else: nc.vector.tensor_copy(
                                qkv_sb[4 + h][:, b * N:(b + 1) * N], av[:])
                            if BSUB >= 2 and b % 4 == 3:
                                rr = rrp.tile([128, N], FP, tag="rr", name="rr")
                                nc.vector.reciprocal_approx_fast(rr[:], rp[:])
                                r_recip.append(rr)
                        # softmax divide for this head (av currently = E @ v, unnormalized)
                        rrb16 = []
                        for g in range(BL // 4 if BSUB >= 4 else 0):
                            rb16 = scp2.tile([128, N], BF, tag="rb16", name="rb16")
                            nc.vector.tensor_copy(rb16[:], r_recip[g][:])
                            rrb16.append(rb16)
                        for b in range(BL if BSUB >= 4 else 0):
                            rr = rrb16[b // 4]
                            rb = 32 * (b % 4)
                            rb_ps = prbc.tile([128, N], FP, tag="rbc", name="rbps")
                            nc.tensor.matmul(
                                rb_ps[:], ones_rb[rb:rb + 1, :], rr[rb:rb + 1, :],
                                tile_position=(rb, 0))
                            nc.vector.tensor_tensor(
                                qkv_sb[4 + h][:, b * N:(b + 1) * N],
                                qkv_sb[4 + h][:, b * N:(b + 1) * N],
                                rb_ps[:], OP.mult)

            if DBG_STOP not in ("A", "B"):
                # ========== Phase C: gelu, proj, BN2 ==========
                with tc.tile_pool(name="ppy", bufs=2, space="PSUM") as ppy, \
                     tc.tile_pool(name="pst", bufs=1, space="PSUM") as pst, \
                     tc.tile_pool(name="yb", bufs=1) as yp, \
                     tc.tile_pool(name="sc3", bufs=4) as scp3:
                    gvsum = statp.tile([128, NH], FP)
                    for h in range(NH):
                        nc.scalar.activation(qkv_sb[4 + h][:], qkv_sb[4 + h][:], AF.Gelu,
                                             scale=alpha1[:, 4 + h:5 + h],
                                             bias=beta1[:, 4 + h:5 + h],
                                             accum_out=gvsum[:, h:h + 1])
                    gvs16 = statp.tile([128, NH], BF)
                    nc.vector.tensor_copy(gvs16[:], gvsum[:])

                    y_sb = yp.tile([128, 48 * C], FP)         # 48 chunks of [<=128, 256]
                    yps_sum = pst.tile([1, C], FP, tag="yst")
                    yps_sq = pst.tile([1, C], FP, tag="ysq")
                    for h in range(NH):
                        nc.tensor.matmul(yps_sum[:], gvs16[:, h:h + 1],
                                         wprojT_sb[:, h * C:(h + 1) * C],
                                         start=(h == 0), stop=(h == NH - 1))
                    nchunks = [(bb, nn) for bb in range(BL) for nn in range(3)]
                    for i, (b, nc3) in enumerate(nchunks):
                        ns = MCS[nc3]
                        py = ppy.tile([128, C], FP, tag="py")
                        for h in range(NH):
                            nc.tensor.matmul(
                                py[0:ns, :],
                                qkv_sb[4 + h][:, b * N + 128 * nc3:
                                              b * N + 128 * nc3 + ns],
                                wprojT_sb[:, h * C:(h + 1) * C],
                                start=(h == 0), stop=(h == NH - 1))
                        ysl = y_sb[0:ns, i * C:(i + 1) * C]
                        nc.vector.tensor_copy(ysl, py[0:ns, :])
                        yq = scp3.tile([128, C], BF, tag="yq")
                        nc.vector.tensor_tensor(yq[0:ns, :], ysl, ysl, OP.mult)
                        nc.tensor.matmul(yps_sq[:], ones_c[0:ns, 0:1], yq[0:ns, :],
                                         start=(i == 0), stop=(i == len(nchunks) - 1))

                    st2 = statp.tile([1, 2 * C], FP)
                    nc.vector.tensor_copy(st2[:, 0:C], yps_sum[:])
                    nc.vector.tensor_copy(st2[:, C:2 * C], yps_sq[:])
                    b2i = dramp.tile([1, 2 * C], FP, tag="b2i")
                    b2o = dramp.tile([1, 2 * C], FP, tag="b2o")
                    nc.sync.dma_start(b2i[:], st2[:])
                    nc.gpsimd.collective_compute(
                        "AllReduce", OP.add,
                        replica_groups=[list(range(NCORES))],
                        ins=[b2i.opt()], outs=[b2o.opt()])
                    st2g = statp.tile([1, 2 * C], FP)
                    nc.sync.dma_start(st2g[:], b2o[:])

                    mean2 = statp.tile([1, C], FP)
                    var2 = statp.tile([1, C], FP)
                    tmp2 = statp.tile([1, C], FP)
                    alpha2 = statp.tile([1, C], FP)
                    beta2 = statp.tile([1, C], FP)
                    nc.vector.tensor_scalar(mean2[:], st2g[:, 0:C], 1.0 / NT, None, OP.mult)
                    nc.vector.tensor_scalar(var2[:], st2g[:, C:2 * C], 1.0 / NT, None,
                                            OP.mult)
                    nc.vector.tensor_tensor(tmp2[:], mean2[:], mean2[:], OP.mult)
                    nc.vector.tensor_tensor(var2[:], var2[:], tmp2[:], OP.subtract)
                    nc.vector.tensor_scalar(var2[:], var2[:], EPS, None, OP.add)
                    nc.scalar.activation(tmp2[:], var2[:], AF.Ln)
                    nc.scalar.activation(var2[:], tmp2[:], AF.Exp, scale=-0.5)  # rstd2
                    nc.vector.tensor_tensor(alpha2[:], g2_sb[:], var2[:], OP.mult)
                    nc.vector.tensor_tensor(beta2[:], mean2[:], alpha2[:], OP.mult)
                    nc.vector.tensor_tensor(beta2[:], b2_sb[:], beta2[:], OP.subtract)

                    a2ps = pst.tile([128, C], FP, tag="yst")
                    b2ps = pst.tile([128, C], FP, tag="ysq")
                    nc.tensor.matmul(a2ps[:], ones_r[0:1, :], alpha2[:])
                    nc.tensor.matmul(b2ps[:], ones_r[0:1, :], beta2[:])
                    a2bc = statp.tile([128, C], FP)
                    b2bc = statp.tile([128, C], FP)
                    nc.vector.tensor_copy(a2bc[:], a2ps[:])
                    nc.vector.tensor_copy(b2bc[:], b2ps[:])

                    for i, (b, nc3) in enumerate(nchunks):
                        ns = MCS[nc3]
                        sl = y_sb[0:ns, i * C:(i + 1) * C]
                        nc.vector.tensor_tensor(sl, sl, a2bc[0:ns, :], OP.mult)
                        nc.vector.tensor_tensor(sl, sl, b2bc[0:ns, :], OP.add)
                        r0 = b * N + 128 * nc3
                        nc.sync.dma_start(y_d[r0:r0 + ns, :], sl)
            if DBG_STOP in ("A", "B"):
                dsrc = qkv_sb[0] if DBG_STOP == "A" else qkv_sb[4]
                for i in range(20):
                    dq = statp.tile([128, C], FP, tag="dq", name="dq", bufs=2)
                    nc.vector.tensor_copy(dq[:], dsrc[:, i * C:(i + 1) * C])
                    nc.sync.dma_start(y_d[i * 128:(i + 1) * 128, :], dq[:])

    nc.compile()
    return nc


_PROG = None


def _get_prog():
    global _PROG
    if _PROG is None:
        _PROG = build_program()
    return _PROG


def _host_prep(x, Wqkv, g1, b1, ab, Wproj, g2, b2, idxs):
    perm = np.empty(H, dtype=np.int64)
    for h in range(NH):
        base = h * (2 * DK + DV)
        perm[DK * h: DK * (h + 1)] = np.arange(base, base + DK)
        perm[NH * DK + DK * h: NH * DK + DK * (h + 1)] = \
            np.arange(base + DK, base + 2 * DK)
        perm[2 * NH * DK + DV * h: 2 * NH * DK + DV * (h + 1)] = \
            np.arange(base + 2 * DK, base + 2 * DK + DV)
    x = np.asarray(x, dtype=np.float32)
    Wqkv = np.asarray(Wqkv, dtype=np.float32)
    wqkvT = np.ascontiguousarray(Wqkv[perm, :].T).astype(ml_dtypes.bfloat16)
    g1c = np.ascontiguousarray(np.asarray(g1, np.float32)[perm].reshape(NHC, 128).T)
    b1c = np.ascontiguousarray(np.asarray(b1, np.float32)[perm].reshape(NHC, 128).T)
    wprojT = np.ascontiguousarray(np.asarray(Wproj, np.float32).T).astype(
        ml_dtypes.bfloat16)                                            # (1024, 256)
    eb = np.exp(np.asarray(ab, np.float32))[:, np.asarray(idxs)].astype(
        ml_dtypes.bfloat16)                                            # (8, 320, 320)
    common = {
        "wqkvT": wqkvT, "wprojT": wprojT, "eb": np.ascontiguousarray(eb),
        "g1c": g1c, "b1c": b1c,
        "g2": np.asarray(g2, np.float32).reshape(1, C),
        "b2": np.asarray(b2, np.float32).reshape(1, C),
        "ident": np.eye(128, dtype=np.float32),
    }
    in_maps = []
    for c in range(NCORES):
        m = dict(common)
        m["x"] = np.ascontiguousarray(x[c * BL:(c + 1) * BL].reshape(R, C))
        in_maps.append(m)
    return in_maps


def _run(in_maps, trace=False):
    nc = _get_prog()
    res = run_bass_kernel_spmd(nc, in_maps, core_ids=list(range(NCORES)),
                               trace=trace)
    out = np.concatenate(
        [np.asarray(res.results[c]["y"]).reshape(BL, N, C) for c in range(NCORES)],
        axis=0)
    return out.astype(np.float32), res


def kernel(**inputs):
    out, _ = _run(_host_prep(**inputs))
    return out


def run_traced(**inputs):
    return _run(_host_prep(**inputs), trace=True)

